# revision 2
# baseline (speedup 1.0000x reference)
"""Trainium2 Bass kernel for a 2-layer GCN (PyG GCNConv semantics) on 8 NeuronCores.

Strategy (dst-sharding, per the sharding hint):
  - nodes sharded 12500/core (padded to 12544 = 98*128 tiles of 128 rows)
  - edges partitioned by destination core; within a core grouped by
    (dst-tile, src-group) and padded to 128-edge chunks
  - per chunk: dma_gather of 128 source rows (fp16) + a one-hot matrix
    P[t,r] = (iota==dst_rel)*dinv[dst] built with one fused tensor_scalar,
    then a PE matmul accumulates agg^T[d,r] into PSUM
  - per dst-tile: agg^T @ W^T flips the orientation back to [row, feat]
  - norm separability: norm_e = dinv[src]*dinv[dst]; the dinv[src] factor is
    pre-scaled into the gather source rows, dinv[dst] rides inside P
  - layer 1 -> AllGather of y1 = dinv*(h2) shards -> layer 2
"""
import sys

sys.path.insert(0, "/opt/trn_rl_repo")

import numpy as np

N = 100000
E = 1600000
D = 128
CORES = 8
S = 12500          # real nodes per core
TPC = 98           # dst tiles per core
SP = TPC * 128     # padded nodes per core (12544)
NP = CORES * SP    # padded global rows (100352)
GRP = 32768        # src-group width (int16 gather-index limit)
NGRP = 4
BLK = 8            # dst tiles per gather block
GMAX = 8192        # max indices per dma_gather instruction (ring capacity)


def _build_schedule(src, dst):
    """Static chunk schedule shared by all cores (SPMD: one instruction
    stream). Returns per-core slot arrays + the chunk/block layout."""
    core = dst // S
    dl = dst % S
    t = dl >> 7
    r = dl & 127
    sp = (src // S) * SP + (src % S)
    g = np.minimum(sp // GRP, NGRP - 1)
    srel = sp - g * GRP

    key = (core * TPC + t) * NGRP + g
    order = np.argsort(key, kind="stable")
    cnt = np.bincount(key, minlength=CORES * TPC * NGRP).reshape(CORES, TPC, NGRP)
    K = -(-cnt.max(0) // 128)  # [TPC, NGRP] chunks per (tile, group)

    # chunk layout order: for b in blocks: for g: for t in b: for k in K[t,g]
    chunk_start = np.zeros((TPC, NGRP), np.int64)
    blocks = []
    nchunks = 0
    for b in range((TPC + BLK - 1) // BLK):
        tiles = list(range(b * BLK, min((b + 1) * BLK, TPC)))
        col = 0
        gathers = []
        tile_chunks = {tt: [] for tt in tiles}
        for gg in range(NGRP):
            c0 = col
            slot0 = nchunks * 128
            for tt in tiles:
                chunk_start[tt, gg] = nchunks
                for _ in range(int(K[tt, gg])):
                    tile_chunks[tt].append((col, nchunks))
                    col += 1
                    nchunks += 1
            # split into <=8192-index instructions: the SWDGE descriptor
            # ring holds 1024 descriptors/direction and a gather needs
            # num_idxs/16+1 — a single too-big instruction deadlocks HW
            c_at = c0
            s_at = slot0
            while c_at < col:
                ncols = min(col - c_at, GMAX // 128)
                gathers.append((gg, c_at, c_at + ncols, s_at, ncols * 128))
                c_at += ncols
                s_at += ncols * 128
        blocks.append(dict(tiles=tiles, C=col, gathers=gathers, chunks=tile_chunks))
    NC = nchunks
    NSLOT = NC * 128

    # per-core slot arrays
    skey = key[order]
    runs = np.flatnonzero(np.diff(skey)) + 1
    starts = np.r_[0, runs]
    lens = np.diff(np.r_[starts, len(skey)])
    pos = np.arange(len(skey)) - np.repeat(starts, lens)
    slot = chunk_start[t[order], g[order]] * 128 + pos

    idx_slot = np.zeros((CORES, NSLOT), np.int16)
    dr_slot = np.full((CORES, NSLOT), -1.0, np.float32)
    wd_slot = np.zeros((CORES, NSLOT), np.float32)
    co = core[order]
    idx_slot[co, slot] = srel[order].astype(np.int16)
    dr_slot[co, slot] = r[order].astype(np.float32)
    return idx_slot, dr_slot, wd_slot, order, co, slot, blocks, NC, NSLOT


def _build_bass(blocks, NC, NSLOT):
    import concourse.bacc as bacc
    import concourse.tile as tile
    import concourse.mybir as mybir

    dt = mybir.dt
    nc = bacc.Bacc("TRN2", target_bir_lowering=False, debug=False, num_devices=CORES)

    xs_in = nc.dram_tensor("xs", [NP, D], dt.float16, kind="ExternalInput")
    w1t_in = nc.dram_tensor("w1t", [D, D], dt.float16, kind="ExternalInput")
    w2t_in = nc.dram_tensor("w2t", [D, D], dt.float16, kind="ExternalInput")
    iota_in = nc.dram_tensor("iota", [128, 128], dt.float16, kind="ExternalInput")
    idx_in = nc.dram_tensor("idx", [128, NSLOT // 16], dt.int16, kind="ExternalInput")
    dr_in = nc.dram_tensor("dr", [128, NC], dt.float32, kind="ExternalInput")
    wd_in = nc.dram_tensor("wd", [128, NC], dt.float32, kind="ExternalInput")
    dinv_in = nc.dram_tensor("dinvcol", [128, TPC], dt.float32, kind="ExternalInput")
    c1d_in = nc.dram_tensor("c1d", [SP, D], dt.float16, kind="ExternalInput")
    c2_in = nc.dram_tensor("c2", [SP, D], dt.float32, kind="ExternalInput")
    out_ext = nc.dram_tensor("out", [SP, D], dt.float32, kind="ExternalOutput")

    GBASE = [i * GRP for i in range(NGRP)]
    GLEN = [min(GRP, NP - i * GRP) for i in range(NGRP)]

    with tile.TileContext(nc) as tc:
        with (
            tc.tile_pool(name="const", bufs=1) as cpool,
            tc.tile_pool(name="mblk", bufs=2) as mpool,
            tc.tile_pool(name="pbuf", bufs=6) as ppool,
            tc.tile_pool(name="gs", bufs=4) as gspool,
            tc.tile_pool(name="ytmp", bufs=4) as ytpool,
            tc.tile_pool(name="cload", bufs=4) as clpool,
            tc.tile_pool(name="psumG", bufs=4, space="PSUM") as pgpool,
            tc.tile_pool(name="psumH", bufs=4, space="PSUM") as phpool,
            tc.tile_pool(name="dram", bufs=1, space="DRAM") as dram_pool,
        ):
            iota_t = cpool.tile([128, 128], dt.float16)
            nc.sync.dma_start(out=iota_t[:], in_=iota_in[:, :])
            w1t_t = cpool.tile([D, D], dt.float16)
            nc.sync.dma_start(out=w1t_t[:], in_=w1t_in[:, :])
            w2t_t = cpool.tile([D, D], dt.float16)
            nc.sync.dma_start(out=w2t_t[:], in_=w2t_in[:, :])
            idx_t = cpool.tile([128, NSLOT // 16], dt.int16)
            nc.sync.dma_start(out=idx_t[:], in_=idx_in[:, :])
            dr_t = cpool.tile([128, NC], dt.float32)
            nc.sync.dma_start(out=dr_t[:], in_=dr_in[:, :])
            wd_t = cpool.tile([128, NC], dt.float32)
            nc.sync.dma_start(out=wd_t[:], in_=wd_in[:, :])
            dinv_t = cpool.tile([128, TPC], dt.float32)
            nc.sync.dma_start(out=dinv_t[:], in_=dinv_in[:, :])

            y1_shard = dram_pool.tile([SP, D], dt.float16)
            y1_full = dram_pool.tile([NP, D], dt.float16)

            def layer(src_dram, wt_t, last):
                for blk in blocks:
                    C = blk["C"]
                    m_t = mpool.tile([128, C, D], dt.float16, tag="m")
                    for gg, c0, c1, slot0, num in blk["gathers"]:
                        nc.gpsimd.dma_gather(
                            m_t[:, c0:c1, :],
                            src_dram[GBASE[gg] : GBASE[gg] + GLEN[gg], :],
                            idx_t[:, slot0 // 16 : (slot0 + num) // 16],
                            num,
                            num,
                            D,
                            single_packet=False,
                        )
                    for tt in blk["tiles"]:
                        chunks = blk["chunks"][tt]
                        psum_g = pgpool.tile([128, 128], dt.float32, space="PSUM")
                        nchk = len(chunks)
                        for i, (col, chid) in enumerate(chunks):
                            p_t = ppool.tile([128, 128], dt.float16, tag="p")
                            nc.any.tensor_scalar(
                                out=p_t[:],
                                in0=iota_t[:],
                                scalar1=dr_t[:, chid : chid + 1],
                                scalar2=wd_t[:, chid : chid + 1],
                                op0=mybir.AluOpType.is_equal,
                                op1=mybir.AluOpType.mult,
                            )
                            nc.tensor.matmul(
                                psum_g[:],
                                lhsT=m_t[:, col, :],
                                rhs=p_t[:],
                                start=(i == 0),
                                stop=(i == nchk - 1),
                            )
                        gs_t = gspool.tile([128, 128], dt.float16, tag="gs")
                        nc.any.tensor_copy(out=gs_t[:], in_=psum_g[:])
                        psum_h = phpool.tile([128, 128], dt.float32, space="PSUM")
                        nc.tensor.matmul(
                            psum_h[:], lhsT=gs_t[:], rhs=wt_t[:], start=True, stop=True
                        )
                        rows = slice(tt * 128, (tt + 1) * 128)
                        if not last:
                            tmp_t = ytpool.tile([128, 128], dt.float16, tag="yt")
                            nc.any.tensor_scalar(
                                out=tmp_t[:],
                                in0=psum_h[:],
                                scalar1=dinv_t[:, tt : tt + 1],
                                scalar2=None,
                                op0=mybir.AluOpType.mult,
                            )
                            c1_t = clpool.tile([128, 128], dt.float16, tag="c1")
                            nc.sync.dma_start(out=c1_t[:], in_=c1d_in[rows, :])
                            y1_t = ytpool.tile([128, 128], dt.float16, tag="y1")
                            nc.any.tensor_tensor(
                                out=y1_t[:],
                                in0=tmp_t[:],
                                in1=c1_t[:],
                                op=mybir.AluOpType.add,
                            )
                            nc.sync.dma_start(out=y1_shard[rows, :], in_=y1_t[:])
                        else:
                            c2_t = clpool.tile([128, 128], dt.float32, tag="c2")
                            nc.sync.dma_start(out=c2_t[:], in_=c2_in[rows, :])
                            o_t = ytpool.tile([128, 128], dt.float32, tag="o")
                            nc.any.tensor_tensor(
                                out=o_t[:],
                                in0=psum_h[:],
                                in1=c2_t[:],
                                op=mybir.AluOpType.add,
                            )
                            nc.sync.dma_start(out=out_ext[rows, :], in_=o_t[:])

            layer(xs_in, w1t_t, last=False)
            nc.gpsimd.collective_compute(
                "AllGather",
                mybir.AluOpType.bypass,
                replica_groups=[list(range(CORES))],
                ins=[y1_shard.opt()],
                outs=[y1_full.opt()],
            )
            layer(y1_full, w2t_t, last=True)

    nc.compile()
    return nc


def _prepare(x, edge_index, perturb_first, perturb_last, W1, b1, W2, b2):
    x = np.asarray(x, np.float32)
    edge_index = np.asarray(edge_index)
    src = np.concatenate([edge_index[0], np.arange(N, dtype=edge_index.dtype)]).astype(
        np.int64
    )
    dst = np.concatenate([edge_index[1], np.arange(N, dtype=edge_index.dtype)]).astype(
        np.int64
    )
    deg = np.bincount(dst, minlength=N).astype(np.float32)
    dinv = 1.0 / np.sqrt(deg)

    idx_slot, dr_slot, wd_slot, order, co, slot, blocks, NC, NSLOT = _build_schedule(
        src, dst
    )
    wd_slot[co, slot] = dinv[dst[order]]

    # gather source: xs = dinv * x, padded to NP rows in shard-major layout
    xs = np.zeros((NP, D), np.float16)
    dinv_x = (dinv[:, None] * x).astype(np.float16)
    for c in range(CORES):
        xs[c * SP : c * SP + S] = dinv_x[c * S : (c + 1) * S]

    iota = np.broadcast_to(np.arange(128, dtype=np.float16), (128, 128)).copy()
    w1t = np.asarray(W1, np.float32).T.astype(np.float16).copy()
    w2t = np.asarray(W2, np.float32).T.astype(np.float16).copy()

    c1 = np.asarray(perturb_first, np.float32) + np.asarray(b1, np.float32)[None, :]
    c1d = dinv[:, None] * c1
    c2 = np.asarray(perturb_last, np.float32) + np.asarray(b2, np.float32)[None, :]

    in_maps = []
    for c in range(CORES):
        rows = slice(c * S, (c + 1) * S)
        c1d_p = np.zeros((SP, D), np.float16)
        c1d_p[:S] = c1d[rows].astype(np.float16)
        c2_p = np.zeros((SP, D), np.float32)
        c2_p[:S] = c2[rows]
        dinvcol = np.zeros((TPC * 128,), np.float32)
        dinvcol[:S] = dinv[rows]
        idx_l = np.tile(idx_slot[c].reshape(-1, 16).T, (8, 1)).copy()
        dr_l = np.ascontiguousarray(dr_slot[c].reshape(NC, 128).T)
        wd_l = np.ascontiguousarray(wd_slot[c].reshape(NC, 128).T)
        in_maps.append(
            {
                "xs": xs,
                "w1t": w1t,
                "w2t": w2t,
                "iota": iota,
                "idx": idx_l,
                "dr": dr_l,
                "wd": wd_l,
                "dinvcol": np.ascontiguousarray(dinvcol.reshape(TPC, 128).T),
                "c1d": c1d_p,
                "c2": c2_p,
            }
        )
    return in_maps, blocks, NC, NSLOT


def kernel(
    x,
    edge_index,
    perturb_first,
    perturb_last,
    W1,
    b1,
    W2,
    b2,
    _results=[],
    _trace=False,
    _tmpdir=None,
):
    from concourse.bass_utils import run_bass_kernel_spmd

    in_maps, blocks, NC, NSLOT = _prepare(
        x, edge_index, perturb_first, perturb_last, W1, b1, W2, b2
    )
    nc = _build_bass(blocks, NC, NSLOT)
    res = run_bass_kernel_spmd(
        nc, in_maps, core_ids=list(range(CORES)), trace=_trace, tmpdir=_tmpdir
    )
    _results.append(res)
    out = np.concatenate([res.results[c]["out"][:S] for c in range(CORES)], axis=0)
    return out.astype(np.float32)



# revision 3
# speedup vs baseline: 1.2018x; 1.2018x over previous
"""Trainium2 Bass kernel for a 2-layer GCN (PyG GCNConv semantics) on 8 NeuronCores.

Strategy (dst-sharding, per the sharding hint):
  - nodes sharded 12500/core (padded to 12544 = 98*128 tiles of 128 rows)
  - edges partitioned by destination core; within a core grouped by
    (dst-tile, src-group) and padded to 128-edge chunks
  - per chunk: dma_gather of 128 source rows (fp16) + a host-precomputed
    one-hot matrix P[t,r] = (slot's dst_rel == r), bulk-loaded via HWDGE,
    then a PE matmul accumulates agg^T[d,r] into PSUM
  - per dst-tile: agg^T @ W^T flips the orientation back to [row, feat];
    dinv[dst] is applied post-matmul on the ACT engine (per-partition scale)
  - norm separability: norm_e = dinv[src]*dinv[dst]; the dinv[src] factor is
    pre-scaled into the gather source rows, dinv[dst] post-applied
  - layer 1 -> AllGather of y1 shards -> layer 2

Engine discipline (the 5.3ms baseline lesson): DVE tensor_scalar/copy ops
enter 2-port perf mode and FULLY BLOCK GpSimd SWDGE descriptor generation
(the gathers). So: no on-device P build (precomputed on host), PSUM
evacuation + scaling on ACT (never contends), adds on DVE tensor_tensor
(single-port, never contends).
"""
import sys

sys.path.insert(0, "/opt/trn_rl_repo")

import numpy as np

N = 100000
E = 1600000
D = 128
CORES = 8
S = 12500          # real nodes per core
TPC = 98           # dst tiles per core
SP = TPC * 128     # padded nodes per core (12544)
NP = CORES * SP    # padded global rows (100352)
GRP = 32768        # src-group width (int16 gather-index limit)
NGRP = 4
BLK = 8            # dst tiles per gather block
GMAX = 8192        # max indices per dma_gather instruction (ring capacity)


def _build_schedule(src, dst):
    """Static chunk schedule shared by all cores (SPMD: one instruction
    stream). Returns per-core slot arrays + the chunk/block layout."""
    core = dst // S
    dl = dst % S
    t = dl >> 7
    r = dl & 127
    sp = (src // S) * SP + (src % S)
    g = np.minimum(sp // GRP, NGRP - 1)
    srel = sp - g * GRP

    key = (core * TPC + t) * NGRP + g
    order = np.argsort(key, kind="stable")
    cnt = np.bincount(key, minlength=CORES * TPC * NGRP).reshape(CORES, TPC, NGRP)
    K = -(-cnt.max(0) // 128)  # [TPC, NGRP] chunks per (tile, group)

    # chunk layout order: for b in blocks: for g: for t in b: for k in K[t,g]
    chunk_start = np.zeros((TPC, NGRP), np.int64)
    blocks = []
    nchunks = 0
    for b in range((TPC + BLK - 1) // BLK):
        tiles = list(range(b * BLK, min((b + 1) * BLK, TPC)))
        col = 0
        gathers = []
        tile_chunks = {tt: [] for tt in tiles}
        for gg in range(NGRP):
            c0 = col
            slot0 = nchunks * 128
            for tt in tiles:
                chunk_start[tt, gg] = nchunks
                for _ in range(int(K[tt, gg])):
                    tile_chunks[tt].append((col, nchunks))
                    col += 1
                    nchunks += 1
            # split into <=8192-index instructions: the SWDGE descriptor
            # ring holds 1024 descriptors/direction and a gather needs
            # num_idxs/16+1 — a single too-big instruction deadlocks HW
            c_at = c0
            s_at = slot0
            while c_at < col:
                ncols = min(col - c_at, GMAX // 128)
                gathers.append((gg, c_at, c_at + ncols, s_at, ncols * 128))
                c_at += ncols
                s_at += ncols * 128
        blocks.append(dict(tiles=tiles, C=col, gathers=gathers, chunks=tile_chunks))
    NC = nchunks
    NSLOT = NC * 128

    # per-core slot arrays
    skey = key[order]
    runs = np.flatnonzero(np.diff(skey)) + 1
    starts = np.r_[0, runs]
    lens = np.diff(np.r_[starts, len(skey)])
    pos = np.arange(len(skey)) - np.repeat(starts, lens)
    slot = chunk_start[t[order], g[order]] * 128 + pos

    idx_slot = np.zeros((CORES, NSLOT), np.int16)
    dr_slot = np.full((CORES, NSLOT), -1, np.int32)
    co = core[order]
    idx_slot[co, slot] = srel[order].astype(np.int16)
    dr_slot[co, slot] = r[order]
    return idx_slot, dr_slot, blocks, NC, NSLOT


def _tile_perm(blocks):
    """Per-tile-contiguous chunk permutation + per-tile chunk offsets."""
    perm = []
    tco = {}
    for blk in blocks:
        for tt in blk["tiles"]:
            tco[tt] = len(perm)
            for _col, chid in blk["chunks"][tt]:
                perm.append(chid)
    return np.array(perm, np.int64), tco


def _build_bass(blocks, NC, NSLOT):
    import concourse.bacc as bacc
    import concourse.tile as tile
    import concourse.mybir as mybir

    dt = mybir.dt
    nc = bacc.Bacc("TRN2", target_bir_lowering=False, debug=False, num_devices=CORES)

    xs_in = nc.dram_tensor("xs", [NP, D], dt.float16, kind="ExternalInput")
    w1t_in = nc.dram_tensor("w1t", [D, D], dt.float16, kind="ExternalInput")
    w2t_in = nc.dram_tensor("w2t", [D, D], dt.float16, kind="ExternalInput")
    idx_in = nc.dram_tensor("idx", [128, NSLOT // 16], dt.int16, kind="ExternalInput")
    pm_in = nc.dram_tensor("pm", [128, NSLOT], dt.float16, kind="ExternalInput")
    d1_in = nc.dram_tensor("d1col", [128, TPC], dt.float32, kind="ExternalInput")
    d2_in = nc.dram_tensor("d2col", [128, TPC], dt.float32, kind="ExternalInput")
    c1d_in = nc.dram_tensor("c1d", [SP, D], dt.float16, kind="ExternalInput")
    c2_in = nc.dram_tensor("c2", [SP, D], dt.float32, kind="ExternalInput")
    out_ext = nc.dram_tensor("out", [SP, D], dt.float32, kind="ExternalOutput")

    GBASE = [i * GRP for i in range(NGRP)]
    GLEN = [min(GRP, NP - i * GRP) for i in range(NGRP)]

    _, tco = _tile_perm(blocks)

    with tile.TileContext(nc) as tc:
        with (
            tc.tile_pool(name="const", bufs=1) as cpool,
            tc.tile_pool(name="mblk", bufs=2) as mpool,
            tc.tile_pool(name="pblk", bufs=3) as ppool,
            tc.tile_pool(name="gs", bufs=4) as gspool,
            tc.tile_pool(name="ytmp", bufs=4) as ytpool,
            tc.tile_pool(name="cload", bufs=4) as clpool,
            tc.tile_pool(name="psumG", bufs=4, space="PSUM") as pgpool,
            tc.tile_pool(name="psumH", bufs=4, space="PSUM") as phpool,
            tc.tile_pool(name="dram", bufs=1, space="DRAM") as dram_pool,
        ):
            w1t_t = cpool.tile([D, D], dt.float16)
            nc.sync.dma_start(out=w1t_t[:], in_=w1t_in[:, :])
            w2t_t = cpool.tile([D, D], dt.float16)
            nc.sync.dma_start(out=w2t_t[:], in_=w2t_in[:, :])
            idx_t = cpool.tile([128, NSLOT // 16], dt.int16)
            nc.sync.dma_start(out=idx_t[:], in_=idx_in[:, :])
            d1_t = cpool.tile([128, TPC], dt.float32)
            nc.sync.dma_start(out=d1_t[:], in_=d1_in[:, :])
            d2_t = cpool.tile([128, TPC], dt.float32)
            nc.sync.dma_start(out=d2_t[:], in_=d2_in[:, :])

            y1_shard = dram_pool.tile([SP, D], dt.float16)
            y1_full = dram_pool.tile([NP, D], dt.float16)

            def layer(src_dram, wt_t, last):
                for blk in blocks:
                    C = blk["C"]
                    m_t = mpool.tile([128, C, D], dt.float16, tag="m")
                    for gg, c0, c1, slot0, num in blk["gathers"]:
                        nc.gpsimd.dma_gather(
                            m_t[:, c0:c1, :],
                            src_dram[GBASE[gg] : GBASE[gg] + GLEN[gg], :],
                            idx_t[:, slot0 // 16 : (slot0 + num) // 16],
                            num,
                            num,
                            D,
                            single_packet=False,
                        )
                    for tt in blk["tiles"]:
                        chunks = blk["chunks"][tt]
                        nchk = len(chunks)
                        p_t = ppool.tile([128, nchk, 128], dt.float16, tag="p")
                        o0 = tco[tt] * 128
                        nc.sync.dma_start(
                            out=p_t[:], in_=pm_in[:, o0 : o0 + nchk * 128]
                        )
                        psum_g = pgpool.tile([128, 128], dt.float32, space="PSUM")
                        for i, (col, _chid) in enumerate(chunks):
                            nc.tensor.matmul(
                                psum_g[:],
                                lhsT=m_t[:, col, :],
                                rhs=p_t[:, i, :],
                                start=(i == 0),
                                stop=(i == nchk - 1),
                            )
                        gs_t = gspool.tile([128, 128], dt.float16, tag="gs")
                        nc.scalar.copy(out=gs_t[:], in_=psum_g[:])
                        psum_h = phpool.tile([128, 128], dt.float32, space="PSUM")
                        nc.tensor.matmul(
                            psum_h[:], lhsT=gs_t[:], rhs=wt_t[:], start=True, stop=True
                        )
                        rows = slice(tt * 128, (tt + 1) * 128)
                        if not last:
                            tmp_t = ytpool.tile([128, 128], dt.float16, tag="yt")
                            nc.scalar.mul(tmp_t[:], psum_h[:], d1_t[:, tt : tt + 1])
                            c1_t = clpool.tile([128, 128], dt.float16, tag="c1")
                            nc.sync.dma_start(out=c1_t[:], in_=c1d_in[rows, :])
                            y1_t = ytpool.tile([128, 128], dt.float16, tag="y1")
                            nc.vector.tensor_tensor(
                                out=y1_t[:],
                                in0=tmp_t[:],
                                in1=c1_t[:],
                                op=mybir.AluOpType.add,
                            )
                            nc.sync.dma_start(out=y1_shard[rows, :], in_=y1_t[:])
                        else:
                            tmp_t = ytpool.tile([128, 128], dt.float32, tag="yt2")
                            nc.scalar.mul(tmp_t[:], psum_h[:], d2_t[:, tt : tt + 1])
                            c2_t = clpool.tile([128, 128], dt.float32, tag="c2")
                            nc.sync.dma_start(out=c2_t[:], in_=c2_in[rows, :])
                            o_t = ytpool.tile([128, 128], dt.float32, tag="o")
                            nc.vector.tensor_tensor(
                                out=o_t[:],
                                in0=tmp_t[:],
                                in1=c2_t[:],
                                op=mybir.AluOpType.add,
                            )
                            nc.sync.dma_start(out=out_ext[rows, :], in_=o_t[:])

            layer(xs_in, w1t_t, last=False)
            nc.gpsimd.collective_compute(
                "AllGather",
                mybir.AluOpType.bypass,
                replica_groups=[list(range(CORES))],
                ins=[y1_shard.opt()],
                outs=[y1_full.opt()],
            )
            layer(y1_full, w2t_t, last=True)

    nc.compile()
    return nc


def _prepare(x, edge_index, perturb_first, perturb_last, W1, b1, W2, b2):
    x = np.asarray(x, np.float32)
    edge_index = np.asarray(edge_index)
    src = np.concatenate([edge_index[0], np.arange(N, dtype=edge_index.dtype)]).astype(
        np.int64
    )
    dst = np.concatenate([edge_index[1], np.arange(N, dtype=edge_index.dtype)]).astype(
        np.int64
    )
    deg = np.bincount(dst, minlength=N).astype(np.float32)
    dinv = 1.0 / np.sqrt(deg)

    idx_slot, dr_slot, blocks, NC, NSLOT = _build_schedule(src, dst)
    perm, _tco = _tile_perm(blocks)

    # gather source: xs = dinv * x, padded to NP rows in shard-major layout
    xs = np.zeros((NP, D), np.float16)
    dinv_x = (dinv[:, None] * x).astype(np.float16)
    for c in range(CORES):
        xs[c * SP : c * SP + S] = dinv_x[c * S : (c + 1) * S]

    w1t = np.asarray(W1, np.float32).T.astype(np.float16).copy()
    w2t = np.asarray(W2, np.float32).T.astype(np.float16).copy()

    c1 = np.asarray(perturb_first, np.float32) + np.asarray(b1, np.float32)[None, :]
    c1d = dinv[:, None] * c1
    c2 = np.asarray(perturb_last, np.float32) + np.asarray(b2, np.float32)[None, :]

    sl = np.arange(NSLOT)
    chunkid = sl >> 7
    slotin = sl & 127

    in_maps = []
    for c in range(CORES):
        rows = slice(c * S, (c + 1) * S)
        c1d_p = np.zeros((SP, D), np.float16)
        c1d_p[:S] = c1d[rows].astype(np.float16)
        c2_p = np.zeros((SP, D), np.float32)
        c2_p[:S] = c2[rows]
        dcol = np.zeros((TPC * 128,), np.float32)
        dcol[:S] = dinv[rows]
        idx_l = np.tile(idx_slot[c].reshape(-1, 16).T, (8, 1)).copy()

        # one-hot P: [chunk, slot, col] -> tile-permuted -> [slot, chunk*col]
        P = np.zeros((NC, 128, 128), np.float16)
        dr = dr_slot[c]
        v = dr >= 0
        P[chunkid[v], slotin[v], dr[v]] = 1.0
        pm = np.ascontiguousarray(
            P[perm].transpose(1, 0, 2).reshape(128, NSLOT)
        )
        in_maps.append(
            {
                "xs": xs,
                "w1t": w1t,
                "w2t": w2t,
                "idx": idx_l,
                "pm": pm,
                "d1col": np.ascontiguousarray((dcol ** 2).reshape(TPC, 128).T),
                "d2col": np.ascontiguousarray(dcol.reshape(TPC, 128).T),
                "c1d": c1d_p,
                "c2": c2_p,
            }
        )
    return in_maps, blocks, NC, NSLOT


def kernel(
    x,
    edge_index,
    perturb_first,
    perturb_last,
    W1,
    b1,
    W2,
    b2,
    _results=[],
    _trace=False,
    _tmpdir=None,
):
    from concourse.bass_utils import run_bass_kernel_spmd

    in_maps, blocks, NC, NSLOT = _prepare(
        x, edge_index, perturb_first, perturb_last, W1, b1, W2, b2
    )
    nc = _build_bass(blocks, NC, NSLOT)
    res = run_bass_kernel_spmd(
        nc, in_maps, core_ids=list(range(CORES)), trace=_trace, tmpdir=_tmpdir
    )
    _results.append(res)
    out = np.concatenate([res.results[c]["out"][:S] for c in range(CORES)], axis=0)
    return out.astype(np.float32)


# revision 4
# speedup vs baseline: 1.4031x; 1.1674x over previous
"""Trainium2 Bass kernel for a 2-layer GCN (PyG GCNConv semantics) on 8 NeuronCores.

v2: cross-tile packed gather spans.

  - nodes sharded 12500/core (12544 padded = 98 tiles of 128 rows); edges
    partitioned by dst core, grouped by (dst-block of 8 tiles, src-group),
    sorted by dst tile within each (block, group) span
  - spans are chunked into 128-slot chunks with NO per-tile alignment: a
    chunk at a tile boundary is consumed by both tiles (each with its own
    host-precomputed one-hot P slice; other tiles' slots are zero rows)
  - per span: ONE dma_gather (<=8192 idx). Pads are trailing: idx=-1 with a
    per-core runtime valid count (num_idxs_reg via value_load), so the Q7
    SWDGE loop never touches them (PAD_NEG=True), or idx=0 gathered rows
    with zero P rows (PAD_NEG=False)
  - per chunk: PE matmul accumulates agg^T[feat, dstrow] into PSUM
  - per dst-tile: agg^T @ W^T; dinv[dst] applied post-matmul on ACT
  - layer 1 -> AllGather of y1 shards -> layer 2 (same schedule, src=y1)

Engine discipline: no DVE tensor_scalar/copy (2-port perf mode blocks the
SWDGE descriptor generation that dominates runtime). PSUM evacuation +
scaling on ACT; adds on DVE tensor_tensor (single-port)."""
import sys

sys.path.insert(0, "/opt/trn_rl_repo")

import numpy as np

N = 100000
E = 1600000
D = 128
CORES = 8
S = 12500          # real nodes per core
TPC = 98           # dst tiles per core
SP = TPC * 128     # padded nodes per core (12544)
NP = CORES * SP    # padded global rows (100352)
GRP = 32768        # src-group width (int16 gather-index limit)
NGRP = 4
BLK = 8            # dst tiles per block
NB = (TPC + BLK - 1) // BLK
PAD_NEG = False    # HW-measured: trailing -1 pads cost MORE gen time than
                   # gathered idx-0 pads, and value_load num_idxs_reg wedges
                   # the device -- so pads are idx 0, gathered, zero P rows
SPLIT = 32         # chunks per gather instruction (4096 idx: no ring stalls)


def _build_schedule(src, dst):
    core = dst // S
    dl = dst % S
    t = dl >> 7
    r = dl & 127
    b = t // BLK
    sp = (src // S) * SP + (src % S)
    g = np.minimum(sp // GRP, NGRP - 1)
    srel = sp - g * GRP

    key = ((core * NB + b) * NGRP + g) * TPC + t
    order = np.argsort(key, kind="stable")
    cbg = (core * NB + b) * NGRP + g
    cnt_cbg = np.bincount(cbg, minlength=CORES * NB * NGRP).reshape(CORES, NB, NGRP)
    CH = -(-cnt_cbg.max(0) // 128)  # [NB, NGRP] chunks per span
    assert CH.max() * 128 <= 8192
    cnt_cbgt = np.bincount(key, minlength=CORES * NB * NGRP * TPC).reshape(
        CORES, NB, NGRP, TPC
    )

    # span chunk base, block column base (span layout: per block, groups 0..3)
    # gathers split at SPLIT chunks so SWDGE desc-gen never stalls on ring
    # space (8192-idx instructions leave ~12us gaps; 2-4k-idx ones do not)
    span_base = np.zeros((NB, NGRP), np.int64)
    nchunks = 0
    nsplits = 0
    blocks = []
    split_span = []  # split -> (b, g, chunk offset within span, nch)
    for bb in range(NB):
        tiles = list(range(bb * BLK, min((bb + 1) * BLK, TPC)))
        col0 = nchunks
        gathers = []
        for gg in range(NGRP):
            span_base[bb, gg] = nchunks
            ch = int(CH[bb, gg])
            at = 0
            while at < ch:
                nch = min(SPLIT, ch - at)
                gathers.append((gg, nchunks - col0 + at, nch, nsplits))
                split_span.append((bb, gg, at, nch))
                nsplits += 1
                at += nch
            nchunks += ch
        blocks.append(dict(tiles=tiles, C=nchunks - col0, gathers=gathers))
    NC = nchunks
    NSLOT = NC * 128

    # static per-tile chunk lists from min/max tile offsets within spans
    tile_chunks = {tt: [] for tt in range(TPC)}  # (col_in_block, g, chunk_in_span)
    for bb in range(NB):
        tiles = blocks[bb]["tiles"]
        col0 = span_base[bb, 0] if NGRP > 0 else 0
        for gg in range(NGRP):
            c = cnt_cbgt[:, bb, gg, tiles]  # [CORES, ntiles]
            off = np.concatenate(
                [np.zeros((CORES, 1), np.int64), np.cumsum(c, axis=1)], axis=1
            )
            for j, tt in enumerate(tiles):
                lo = int(off[:, j].min()) >> 7
                hi = -(-int(off[:, j + 1].max()) // 128)
                hi = min(hi, int(CH[bb, gg]))
                for k in range(lo, hi):
                    tile_chunks[tt].append(
                        (int(span_base[bb, gg] - col0) + k, gg, k)
                    )

    # per-core slot arrays
    skey = key[order]
    runs = np.flatnonzero(np.diff(skey)) + 1
    starts = np.r_[0, runs]
    lens = np.diff(np.r_[starts, len(skey)])
    # position within the (c,b,g) span: position within run + offset of run's
    # tile within the span
    off_in_span = np.zeros(len(skey), np.int64)
    co_sorted = core[order]
    b_sorted = b[order]
    g_sorted = g[order]
    t_sorted = t[order]
    run_first = starts
    run_tile_off = np.zeros(len(starts), np.int64)
    for i, st in enumerate(starts):
        c_, b_, g_, t_ = (
            co_sorted[st],
            b_sorted[st],
            g_sorted[st],
            t_sorted[st],
        )
        tiles = blocks[b_]["tiles"]
        j = t_ - tiles[0]
        run_tile_off[i] = cnt_cbgt[c_, b_, g_, tiles[0] : tiles[0] + j].sum()
    pos_in_run = np.arange(len(skey)) - np.repeat(starts, lens)
    pos_in_span = pos_in_run + np.repeat(run_tile_off, lens)
    slot = span_base[b_sorted, g_sorted] * 128 + pos_in_span

    pad_val = -1 if PAD_NEG else 0
    idx_slot = np.full((CORES, NSLOT), pad_val, np.int16)
    dr_slot = np.full((CORES, NSLOT), -1, np.int32)
    t_slot = np.full((CORES, NSLOT), -1, np.int32)
    idx_slot[co_sorted, slot] = srel[order].astype(np.int16)
    dr_slot[co_sorted, slot] = r[order]
    t_slot[co_sorted, slot] = t_sorted

    # per-(core, split) valid counts; pads are trailing per span, so a
    # split's valid count is a clamp of (span count - split base)
    cnt_span = cnt_cbg.reshape(CORES, NB * NGRP)
    if PAD_NEG:
        # first two blocks of the program gather their pads (idx 0) so the
        # m-tile double buffers never expose uninitialized SBUF to the PE
        for bb in range(min(2, NB)):
            for gg in range(NGRP):
                s0 = int(span_base[bb, gg]) * 128
                s1 = s0 + int(CH[bb, gg]) * 128
                m = np.zeros(NSLOT, bool)
                m[s0:s1] = True
                idx_slot[m[None, :] & (idx_slot < 0)] = 0
                cnt_span[:, bb * NGRP + gg] = int(CH[bb, gg]) * 128
    nvalid = np.zeros((CORES, len(split_span)), np.int32)
    for si, (bb, gg, at, nch) in enumerate(split_span):
        base = at * 128
        v = np.clip(cnt_span[:, bb * NGRP + gg] - base, 0, nch * 128)
        if PAD_NEG:
            # a core with an all-pad split still needs >=1 valid idx: make
            # the split's first slot a gathered idx-0 pad (P row is zero)
            z = v == 0
            if z.any():
                s0 = (int(span_base[bb, gg]) + at) * 128
                idx_slot[z, s0] = 0
                v = np.maximum(v, 1)
        nvalid[:, si] = v
    return idx_slot, dr_slot, t_slot, nvalid, blocks, tile_chunks, CH, span_base, NC, NSLOT


def _build_bass(blocks, tile_chunks, CH, NC, NSLOT, NPCH):
    import concourse.bacc as bacc
    import concourse.tile as tile
    import concourse.mybir as mybir

    dt = mybir.dt
    nc = bacc.Bacc("TRN2", target_bir_lowering=False, debug=False, num_devices=CORES)

    xs_in = nc.dram_tensor("xs", [NP, D], dt.float16, kind="ExternalInput")
    w1t_in = nc.dram_tensor("w1t", [D, D], dt.float16, kind="ExternalInput")
    w2t_in = nc.dram_tensor("w2t", [D, D], dt.float16, kind="ExternalInput")
    idx_in = nc.dram_tensor("idx", [128, NSLOT // 16], dt.int16, kind="ExternalInput")
    pm_in = nc.dram_tensor("pm", [128, NPCH * 128], dt.float16, kind="ExternalInput")
    nv_in = nc.dram_tensor("nv", [1, NB * NGRP], dt.int32, kind="ExternalInput")
    d1_in = nc.dram_tensor("d1col", [128, TPC], dt.float32, kind="ExternalInput")
    d2_in = nc.dram_tensor("d2col", [128, TPC], dt.float32, kind="ExternalInput")
    c1d_in = nc.dram_tensor("c1d", [SP, D], dt.float16, kind="ExternalInput")
    c2_in = nc.dram_tensor("c2", [SP, D], dt.float32, kind="ExternalInput")
    out_ext = nc.dram_tensor("out", [SP, D], dt.float32, kind="ExternalOutput")

    GBASE = [i * GRP for i in range(NGRP)]
    GLEN = [min(GRP, NP - i * GRP) for i in range(NGRP)]

    # per-tile P offset (tile-major contiguous pidx)
    pofs = {}
    acc = 0
    for tt in range(TPC):
        pofs[tt] = acc
        acc += len(tile_chunks[tt])
    assert acc == NPCH

    with tile.TileContext(nc) as tc:
        with (
            tc.tile_pool(name="const", bufs=1) as cpool,
            tc.tile_pool(name="mblk", bufs=2) as mpool,
            tc.tile_pool(name="pblk", bufs=3) as ppool,
            tc.tile_pool(name="gs", bufs=4) as gspool,
            tc.tile_pool(name="ytmp", bufs=4) as ytpool,
            tc.tile_pool(name="cload", bufs=4) as clpool,
            tc.tile_pool(name="psumG", bufs=4, space="PSUM") as pgpool,
            tc.tile_pool(name="psumH", bufs=4, space="PSUM") as phpool,
            tc.tile_pool(name="dram", bufs=1, space="DRAM") as dram_pool,
        ):
            w1t_t = cpool.tile([D, D], dt.float16)
            nc.sync.dma_start(out=w1t_t[:], in_=w1t_in[:, :])
            w2t_t = cpool.tile([D, D], dt.float16)
            nc.sync.dma_start(out=w2t_t[:], in_=w2t_in[:, :])
            idx_t = cpool.tile([128, NSLOT // 16], dt.int16)
            nc.sync.dma_start(out=idx_t[:], in_=idx_in[:, :])
            nv_t = cpool.tile([1, NB * NGRP], dt.int32)
            nc.sync.dma_start(out=nv_t[:], in_=nv_in[:, :])
            d1_t = cpool.tile([128, TPC], dt.float32)
            nc.sync.dma_start(out=d1_t[:], in_=d1_in[:, :])
            d2_t = cpool.tile([128, TPC], dt.float32)
            nc.sync.dma_start(out=d2_t[:], in_=d2_in[:, :])

            y1_shard = dram_pool.tile([SP, D], dt.float16)
            y1_full = dram_pool.tile([NP, D], dt.float16)

            def layer(src_dram, wt_t, last):
                for bb, blk in enumerate(blocks):
                    C = blk["C"]
                    m_t = mpool.tile([128, C, D], dt.float16, tag="m")
                    for gg, cofs, nch, _spl in blk["gathers"]:
                        if nch == 0:
                            continue
                        # m_t columns [cofs, cofs+nch); pads are idx 0 so
                        # every slot is gathered (num_idxs_reg == num)
                        num = nch * 128
                        s0 = (sum(b2["C"] for b2 in blocks[:bb]) + cofs) * 128
                        nc.gpsimd.dma_gather(
                            m_t[:, cofs : cofs + nch, :],
                            src_dram[GBASE[gg] : GBASE[gg] + GLEN[gg], :],
                            idx_t[:, s0 // 16 : (s0 + num) // 16],
                            num,
                            num,
                            D,
                            single_packet=False,
                        )
                    for tt in blk["tiles"]:
                        chunks = tile_chunks[tt]
                        nchk = len(chunks)
                        p_t = ppool.tile([128, nchk, 128], dt.float16, tag="p")
                        o0 = pofs[tt] * 128
                        nc.sync.dma_start(
                            out=p_t[:], in_=pm_in[:, o0 : o0 + nchk * 128]
                        )
                        psum_g = pgpool.tile([128, 128], dt.float32, space="PSUM")
                        for i, (col, _gg, _k) in enumerate(chunks):
                            nc.tensor.matmul(
                                psum_g[:],
                                lhsT=m_t[:, col, :],
                                rhs=p_t[:, i, :],
                                start=(i == 0),
                                stop=(i == nchk - 1),
                            )
                        gs_t = gspool.tile([128, 128], dt.float16, tag="gs")
                        nc.scalar.copy(out=gs_t[:], in_=psum_g[:])
                        psum_h = phpool.tile([128, 128], dt.float32, space="PSUM")
                        nc.tensor.matmul(
                            psum_h[:], lhsT=gs_t[:], rhs=wt_t[:], start=True, stop=True
                        )
                        rows = slice(tt * 128, (tt + 1) * 128)
                        if not last:
                            tmp_t = ytpool.tile([128, 128], dt.float16, tag="yt")
                            nc.scalar.mul(tmp_t[:], psum_h[:], d1_t[:, tt : tt + 1])
                            c1_t = clpool.tile([128, 128], dt.float16, tag="c1")
                            nc.sync.dma_start(out=c1_t[:], in_=c1d_in[rows, :])
                            y1_t = ytpool.tile([128, 128], dt.float16, tag="y1")
                            nc.vector.tensor_tensor(
                                out=y1_t[:],
                                in0=tmp_t[:],
                                in1=c1_t[:],
                                op=mybir.AluOpType.add,
                            )
                            nc.sync.dma_start(out=y1_shard[rows, :], in_=y1_t[:])
                        else:
                            tmp_t = ytpool.tile([128, 128], dt.float32, tag="yt2")
                            nc.scalar.mul(tmp_t[:], psum_h[:], d2_t[:, tt : tt + 1])
                            c2_t = clpool.tile([128, 128], dt.float32, tag="c2")
                            nc.sync.dma_start(out=c2_t[:], in_=c2_in[rows, :])
                            o_t = ytpool.tile([128, 128], dt.float32, tag="o")
                            nc.vector.tensor_tensor(
                                out=o_t[:],
                                in0=tmp_t[:],
                                in1=c2_t[:],
                                op=mybir.AluOpType.add,
                            )
                            nc.sync.dma_start(out=out_ext[rows, :], in_=o_t[:])

            layer(xs_in, w1t_t, last=False)
            nc.gpsimd.collective_compute(
                "AllGather",
                mybir.AluOpType.bypass,
                replica_groups=[list(range(CORES))],
                ins=[y1_shard.opt()],
                outs=[y1_full.opt()],
            )
            layer(y1_full, w2t_t, last=True)

    nc.compile()
    return nc


def _prepare(x, edge_index, perturb_first, perturb_last, W1, b1, W2, b2):
    x = np.asarray(x, np.float32)
    edge_index = np.asarray(edge_index)
    src = np.concatenate([edge_index[0], np.arange(N, dtype=edge_index.dtype)]).astype(
        np.int64
    )
    dst = np.concatenate([edge_index[1], np.arange(N, dtype=edge_index.dtype)]).astype(
        np.int64
    )
    deg = np.bincount(dst, minlength=N).astype(np.float32)
    dinv = 1.0 / np.sqrt(deg)

    (
        idx_slot,
        dr_slot,
        t_slot,
        nvalid,
        blocks,
        tile_chunks,
        CH,
        span_base,
        NC,
        NSLOT,
    ) = _build_schedule(src, dst)

    NPCH = sum(len(tile_chunks[tt]) for tt in range(TPC))

    # (tile, g, chunk_in_span) -> pidx
    M = np.full((TPC, NGRP, int(CH.max())), -1, np.int64)
    acc = 0
    for tt in range(TPC):
        for (col, gg, k) in tile_chunks[tt]:
            M[tt, gg, k] = acc
            acc += 1

    xs = np.zeros((NP, D), np.float16)
    dinv_x = (dinv[:, None] * x).astype(np.float16)
    for c in range(CORES):
        xs[c * SP : c * SP + S] = dinv_x[c * S : (c + 1) * S]

    w1t = np.asarray(W1, np.float32).T.astype(np.float16).copy()
    w2t = np.asarray(W2, np.float32).T.astype(np.float16).copy()

    c1 = np.asarray(perturb_first, np.float32) + np.asarray(b1, np.float32)[None, :]
    c1d = dinv[:, None] * c1
    c2 = np.asarray(perturb_last, np.float32) + np.asarray(b2, np.float32)[None, :]

    sl = np.arange(NSLOT)
    chunkid = sl >> 7
    slotin = sl & 127
    # chunk -> (g, chunk_in_span) lookup
    ch_g = np.zeros(NC, np.int64)
    ch_k = np.zeros(NC, np.int64)
    for bb in range(NB):
        for gg in range(NGRP):
            b0 = int(span_base[bb, gg])
            n = int(CH[bb, gg])
            ch_g[b0 : b0 + n] = gg
            ch_k[b0 : b0 + n] = np.arange(n)

    in_maps = []
    for c in range(CORES):
        rows = slice(c * S, (c + 1) * S)
        c1d_p = np.zeros((SP, D), np.float16)
        c1d_p[:S] = c1d[rows].astype(np.float16)
        c2_p = np.zeros((SP, D), np.float32)
        c2_p[:S] = c2[rows]
        dcol = np.zeros((TPC * 128,), np.float32)
        dcol[:S] = dinv[rows]
        idx_l = np.tile(idx_slot[c].reshape(-1, 16).T, (8, 1)).copy()

        P = np.zeros((NPCH, 128, 128), np.float16)
        dr = dr_slot[c]
        ts = t_slot[c]
        v = dr >= 0
        pidx = M[ts[v], ch_g[chunkid[v]], ch_k[chunkid[v]]]
        assert (pidx >= 0).all()
        P[pidx, slotin[v], dr[v]] = 1.0
        pm = np.ascontiguousarray(P.transpose(1, 0, 2).reshape(128, NPCH * 128))
        in_maps.append(
            {
                "xs": xs,
                "w1t": w1t,
                "w2t": w2t,
                "idx": idx_l,
                "pm": pm,
                "nv": nvalid[c : c + 1],
                "d1col": np.ascontiguousarray((dcol ** 2).reshape(TPC, 128).T),
                "d2col": np.ascontiguousarray(dcol.reshape(TPC, 128).T),
                "c1d": c1d_p,
                "c2": c2_p,
            }
        )
    return in_maps, blocks, tile_chunks, CH, NC, NSLOT, NPCH


def kernel(
    x,
    edge_index,
    perturb_first,
    perturb_last,
    W1,
    b1,
    W2,
    b2,
    _results=[],
    _trace=False,
    _tmpdir=None,
):
    from concourse.bass_utils import run_bass_kernel_spmd

    in_maps, blocks, tile_chunks, CH, NC, NSLOT, NPCH = _prepare(
        x, edge_index, perturb_first, perturb_last, W1, b1, W2, b2
    )
    nc = _build_bass(blocks, tile_chunks, CH, NC, NSLOT, NPCH)
    res = run_bass_kernel_spmd(
        nc, in_maps, core_ids=list(range(CORES)), trace=_trace, tmpdir=_tmpdir
    )
    _results.append(res)
    out = np.concatenate([res.results[c]["out"][:S] for c in range(CORES)], axis=0)
    return out.astype(np.float32)
